# revision 1
# baseline (speedup 1.0000x reference)
"""Trainium2 Bass kernel for nn_BasicBlock_HMU (two HMU layers + sync BN + residual).

Sharding: data-parallel over batch (8 cores x 512 rows); mu/lam/v replicated.
BN batch statistics are all-reduced across the 8 cores (sync BN).

Heavy lifting happens in fp32r (TF32-like) matmuls at full PE rate. All
parameter-side transposes are done on the host; the only on-chip transpose is
h (layer-1 output) via the PE (exact in fp32).

Per core, for each layer (orientation: batch on partitions, units on free):
  quad[b,n] = lam_n*|x_b|^2 + lam_n*|mu_n|^2 - 2*lam_n*(x.mu) + sum_k (x.v_k - mu.v_k)^2
computed as PE accumulation groups over a packed weight matrix
  W = [[(-2*lam*mu)^T | v^T], [c1 | -v.mu]]  (1025 x 5120)
with the constants row and the rank-1 lam_n*|x|^2_b term folded in as K=1
matmuls.  z = exp(-quad/D) - 1 (shifted for numerically safe batch stats),
stats = ones-vector matmuls, AllReduce(8KB), normalize, and either PE-transpose
h for layer 2 or add the residual and store.
"""

import numpy as np

import concourse.bacc as bacc
import concourse.mybir as mybir
import concourse.tile as tile

try:
    from concourse.bass_utils import run_bass_kernel_spmd
except ImportError:  # pragma: no cover
    from bass_utils import run_bass_kernel_spmd

F32 = mybir.dt.float32
F32R = mybir.dt.float32r
Alu = mybir.AluOpType
Act = mybir.ActivationFunctionType

N_CORES = 8
B, D, N, K = 4096, 1024, 1024, 4
BS = B // N_CORES          # 512 rows per core
NBT = BS // 128            # 4 batch tiles per core
CH = D // 128              # 8 contraction chunks
NB_MU = N // 512           # 2 moving blocks for the mu matmul
NB_V = (N * K) // 512      # 8 moving blocks for the v matmul
NB_TOT = NB_MU + NB_V      # 10
WROWS = D + 1              # 1024 weight rows + 1 constants row
WCOLS = N + N * K          # 5120
BN_EPS = 1e-5

_CACHE = {}


def _build_nc(reps=1, loop_reps=0, collectives=True):
    nc = bacc.Bacc("TRN2", target_bir_lowering=False, debug=False,
                   num_devices=N_CORES)

    xT_s = nc.dram_tensor("xT_s", [D, BS], F32, kind="ExternalInput").ap()
    xn_s = nc.dram_tensor("xn_s", [BS, N], F32, kind="ExternalInput").ap()
    xsq_s = nc.dram_tensor("xsq_s", [1, BS], F32, kind="ExternalInput").ap()
    W1 = nc.dram_tensor("W1", [WROWS, WCOLS], F32, kind="ExternalInput").ap()
    W2 = nc.dram_tensor("W2", [WROWS, WCOLS], F32, kind="ExternalInput").ap()
    lam = nc.dram_tensor("lam", [2, N], F32, kind="ExternalInput").ap()
    gb = nc.dram_tensor("gb", [4, N], F32, kind="ExternalInput").ap()
    cst = nc.dram_tensor("cst", [128, 128], F32, kind="ExternalInput").ap()
    ones_r = nc.dram_tensor("ones_r", [1, 128], F32, kind="ExternalInput").ap()
    ones_c = nc.dram_tensor("ones_c", [128, 1], F32, kind="ExternalInput").ap()
    out = nc.dram_tensor("out", [BS, N], F32, kind="ExternalOutput").ap()

    with tile.TileContext(nc) as tc:
        with (
            tc.tile_pool(name="const", bufs=1) as constp,
            tc.tile_pool(name="big", bufs=1) as bigp,
            tc.tile_pool(name="wp", bufs=2) as wp,
            tc.tile_pool(name="scr", bufs=2) as scr,
            tc.tile_pool(name="rowp", bufs=1) as rowp,
            tc.tile_pool(name="pmm", bufs=4, space="PSUM") as pmm,
            tc.tile_pool(name="pst", bufs=2, space="PSUM") as pst,
            tc.tile_pool(name="ptr", bufs=2, space="PSUM") as ptr,
            tc.tile_pool(name="dram", bufs=2, space="DRAM") as dramp,
        ):
            # ---- constants / small inputs ----
            ident = constp.tile([128, 128], F32)
            nc.scalar.dma_start(ident[:], cst)
            ones_k1 = constp.tile([1, 128], F32R)
            nc.scalar.dma_start(ones_k1[:], ones_r.bitcast(F32R))
            onec_f32 = constp.tile([128, 1], F32)
            nc.scalar.dma_start(onec_f32[:], ones_c)
            onec_f32r = constp.tile([128, 1], F32R)
            nc.scalar.dma_start(onec_f32r[:], ones_c.bitcast(F32R))

            xsqr = constp.tile([1, BS], F32R)          # |x_b|^2 as a row
            nc.scalar.dma_start(xsqr[:], xsq_s.bitcast(F32R))
            lam1t = constp.tile([1, N], F32R)          # lam rows (K=1 rhs)
            nc.scalar.dma_start(lam1t[:], lam[0:1, :].bitcast(F32R))
            lam2t = constp.tile([1, N], F32R)
            nc.scalar.dma_start(lam2t[:], lam[1:2, :].bitcast(F32R))
            g1t = constp.tile([1, N], F32)
            nc.scalar.dma_start(g1t[:], gb[0:1, :])
            b1t = constp.tile([1, N], F32)
            nc.scalar.dma_start(b1t[:], gb[1:2, :])
            g2t = constp.tile([1, N], F32)
            nc.scalar.dma_start(g2t[:], gb[2:3, :])
            b2t = constp.tile([1, N], F32)
            nc.scalar.dma_start(b2t[:], gb[3:4, :])

            # ---- resident big tiles ----
            hT = bigp.tile([128, CH * BS], F32R, tag="hT")   # h^T for layer 2

            hsqr = rowp.tile([1, BS], F32R, tag="hsqr")      # |h_b|^2 row (layer 2)
            epsc = constp.tile([1, 1], F32)
            nc.gpsimd.memset(epsc[:], BN_EPS)
            sq_warm = constp.tile([1, 1], F32)

            def body():
              # x^T and (later, same slot) x natural; reloaded per rep
              xt = bigp.tile([128, CH * BS], F32R, tag="xt")
              for c in range(CH):
                  nc.scalar.dma_start(xt[:, c * BS:(c + 1) * BS],
                                      xT_s[c * 128:(c + 1) * 128, :].bitcast(F32R))
              xn = None
              for L in range(2):
                  W = (W1, W2)[L]
                  lhsT = (xt, hT)[L]
                  sq_row = (xsqr, hsqr)[L]
                  lam_row = (lam1t, lam2t)[L][:]
                  g_row = (g1t, g2t)[L][:]
                  beta_row = (b1t, b2t)[L][:]

                  if L == 1:
                      # |h_b|^2 row via ones-matmul over h^T chunks
                      ph = pst.tile([1, 512], F32, tag="ps")
                      for c in range(CH):
                          hq = scr.tile([128, 512], F32R, tag="zq")
                          nc.scalar.activation(hq[:], hT[:, c * BS:(c + 1) * BS],
                                               Act.Square)
                          nc.tensor.matmul(ph[:], onec_f32r[:], hq[:],
                                           start=(c == 0), stop=(c == CH - 1))
                      nc.vector.tensor_copy(hsqr[:], ph[:])

                  q = bigp.tile([128, NBT * N], F32, tag="q")   # quad, then exp
                  z = bigp.tile([128, NBT * N], F32R, tag="z")  # z_m1 = exp(.)-1

                  # ---- matmul sweep ----
                  for nb in range(NB_TOT):
                      w = wp.tile([128, CH * 512], F32R, tag="w")
                      wc = wp.tile([1, 512], F32R, tag="wc")
                      for c in range(CH):
                          nc.sync.dma_start(
                              w[:, c * 512:(c + 1) * 512],
                              W[c * 128:(c + 1) * 128,
                                nb * 512:(nb + 1) * 512].bitcast(F32R))
                      nc.scalar.dma_start(
                          wc[:], W[D:D + 1, nb * 512:(nb + 1) * 512].bitcast(F32R))
                      for bt in range(NBT):
                          pm = pmm.tile([128, 512], F32, tag="pm")
                          for c in range(CH):
                              nc.tensor.matmul(
                                  pm[:],
                                  lhsT[:, c * BS + bt * 128:c * BS + (bt + 1) * 128],
                                  w[:, c * 512:(c + 1) * 512],
                                  start=(c == 0), stop=False)
                          if nb < NB_MU:
                              # constants row, then rank-1 lam_n * |x_b|^2
                              nc.tensor.matmul(pm[:], ones_k1[:], wc[:],
                                               start=False, stop=False)
                              nc.tensor.matmul(
                                  pm[:], sq_row[:, bt * 128:(bt + 1) * 128],
                                  lam_row[:, nb * 512:(nb + 1) * 512],
                                  start=False, stop=True)
                              nc.vector.tensor_copy(
                                  q[:, bt * N + nb * 512: bt * N + (nb + 1) * 512],
                                  pm[:])
                          else:
                              nc.tensor.matmul(pm[:], ones_k1[:], wc[:],
                                               start=False, stop=True)
                              # z[:, n-block] += sum_k proj^2
                              nv = nb - NB_MU
                              sqv = scr.tile([128, 512], F32, tag="sqv", bufs=3)
                              nc.scalar.activation(sqv[:], pm[:], Act.Square)
                              pscr = scr.tile([128, 128], F32, tag="pscr", bufs=4)
                              nc.vector.tensor_reduce(
                                  out=pscr[:],
                                  in_=sqv[:].rearrange("p (n k) -> p n k", k=K),
                                  axis=mybir.AxisListType.X,
                                  op=Alu.add)
                              ql = q[:, bt * N + nv * 128: bt * N + (nv + 1) * 128]
                              nc.vector.tensor_tensor(
                                  out=ql, in0=ql, in1=pscr[:], op=Alu.add)

                  if L == 0:
                      xn = bigp.tile([128, NBT * N], F32, tag="xt")
                      nc.scalar.dma_start(
                          xn[:].rearrange("p (t n) -> p t n", n=N),
                          xn_s.rearrange("(t p) n -> p t n", p=128))

                  # ---- quad -> z = exp(-q/D) - 1 ----
                  for bt in range(NBT):
                      for h2 in range(2):
                          sl = slice(bt * N + h2 * 512, bt * N + (h2 + 1) * 512)
                          nc.scalar.activation(q[:, sl], q[:, sl], Act.Exp,
                                               scale=-1.0 / D)
                          nc.vector.tensor_scalar(
                              out=z[:, sl], in0=q[:, sl], scalar1=1.0, scalar2=None,
                              op0=Alu.subtract)

                  # ---- BN stats: S1 = sum_b z, S2 = sum_b z^2 (per n) ----
                  cin = dramp.tile([1, 2 * N], F32, tag="cin")
                  stats = rowp.tile([1, 2 * N], F32, tag="rows")
                  for h2 in range(2):
                      ps1 = pst.tile([1, 512], F32, tag="ps")
                      for bt in range(NBT):
                          sl = slice(bt * N + h2 * 512, bt * N + (h2 + 1) * 512)
                          nc.tensor.matmul(ps1[:], onec_f32r[:], z[:, sl],
                                           start=(bt == 0), stop=(bt == NBT - 1))
                      nc.vector.tensor_copy(stats[:, h2 * 512:(h2 + 1) * 512], ps1[:])
                      ps2 = pst.tile([1, 512], F32, tag="ps")
                      for bt in range(NBT):
                          sl = slice(bt * N + h2 * 512, bt * N + (h2 + 1) * 512)
                          zq = scr.tile([128, 512], F32R, tag="zq")
                          nc.scalar.activation(zq[:], z[:, sl], Act.Square)
                          nc.tensor.matmul(ps2[:], onec_f32r[:], zq[:],
                                           start=(bt == 0), stop=(bt == NBT - 1))
                      nc.scalar.copy(
                          stats[:, N + h2 * 512: N + (h2 + 1) * 512], ps2[:])
                  nc.scalar.dma_start(cin[:], stats[:])

                  # ---- sync-BN: AllGather + on-chip matmul-reduce ----
                  # (AllGather is ~7x cheaper than AllReduce on this stack)
                  cout = dramp.tile([N_CORES, 2 * N], F32, tag="cout",
                                    addr_space="Shared")
                  if collectives:
                      nc.gpsimd.collective_compute(
                          "AllGather", Alu.bypass,
                          replica_groups=[list(range(N_CORES))],
                          ins=[cin[:].opt()], outs=[cout[:].opt()])
                  else:
                      nc.sync.dma_start(cout[0:1, :], cin[:])
                  gath = rowp.tile([N_CORES, 2 * N], F32R, tag="gath")
                  nc.scalar.dma_start(gath[:], cout[:].bitcast(F32R))
                  sred = rowp.tile([1, 2 * N], F32, tag="sred")
                  s2ps = []
                  for j in range(4):
                      js = slice(j * 512, (j + 1) * 512)
                      ps_r = pst.tile([1, 512], F32, tag="ps")
                      nc.tensor.matmul(ps_r[:], onec_f32r[0:N_CORES, :],
                                       gath[:, js], start=True, stop=True)
                      if j < 2:
                          nc.vector.tensor_copy(sred[:, js], ps_r[:])
                      else:
                          s2ps.append(ps_r)
                  # preload the ACT Sqrt table off the critical path
                  nc.scalar.activation(sq_warm[:], epsc[:], Act.Sqrt)

                  # ---- finalize: rows = [sg | beta - m*sg] ----
                  rows = rowp.tile([1, 2 * N], F32, tag="rows")
                  rows_b = rowp.tile([128, 2 * N], F32, tag="rows_b")
                  mrow = rowp.tile([1, N], F32, tag="mrow")
                  for h2 in range(2):
                      hs = slice(h2 * 512, (h2 + 1) * 512)
                      fin = scr.tile([1, 1024], F32, tag="fin", bufs=2)
                      fa, fb = fin[:, 0:512], fin[:, 512:1024]
                      msq, veps, sd, rs, msg = fa, fb, fa, fb, fa
                      m = mrow[:, hs]
                      nc.vector.tensor_scalar(out=m, in0=sred[:, hs],
                                              scalar1=1.0 / B, scalar2=None,
                                              op0=Alu.mult)
                      nc.vector.tensor_tensor(out=msq, in0=m, in1=m, op=Alu.mult)
                      nc.vector.scalar_tensor_tensor(
                          out=veps, in0=s2ps[h2][:],
                          scalar=1.0 / B, in1=msq,
                          op0=Alu.mult, op1=Alu.subtract)
                      nc.scalar.activation(sd, veps, Act.Sqrt, bias=epsc[:])
                      nc.vector.reciprocal(rs, sd)
                      nc.vector.tensor_tensor(out=rows[:, hs], in0=rs,
                                              in1=g_row[:, hs], op=Alu.mult)
                      nc.vector.tensor_tensor(out=msg, in0=m, in1=rows[:, hs],
                                              op=Alu.mult)
                      nc.vector.scalar_tensor_tensor(
                          out=rows[:, N + h2 * 512: N + (h2 + 1) * 512],
                          in0=msg, scalar=-1.0, in1=beta_row[:, hs],
                          op0=Alu.mult, op1=Alu.add)
                      nc.gpsimd.partition_broadcast(
                          rows_b[:, hs], rows[:, hs])
                      nc.gpsimd.partition_broadcast(
                          rows_b[:, N + h2 * 512: N + (h2 + 1) * 512],
                          rows[:, N + h2 * 512: N + (h2 + 1) * 512])


                  # ---- normalize (+ transpose h | + residual & store) ----
                  for bt in range(NBT):
                      if L == 0:
                          hn = scr.tile([128, N], F32, tag="hn", bufs=3)
                          for h2 in range(2):
                              sl = slice(bt * N + h2 * 512, bt * N + (h2 + 1) * 512)
                              hs = slice(h2 * 512, (h2 + 1) * 512)
                              eng = nc.gpsimd if (bt, h2) in ((2, 1), (3, 1)) else nc.vector
                              eng.tensor_tensor(
                                  out=hn[:, hs], in0=z[:, sl], in1=rows_b[:, hs],
                                  op=Alu.mult)
                              eng.tensor_tensor(
                                  out=hn[:, hs], in0=hn[:, hs],
                                  in1=rows_b[:, N + h2 * 512: N + (h2 + 1) * 512],
                                  op=Alu.add)
                          for c in range(CH):
                              pt = ptr.tile([128, 128], F32, tag="pt")
                              nc.tensor.transpose(
                                  pt[:], hn[:, c * 128:(c + 1) * 128], ident[:])
                              nc.scalar.copy(
                                  hT[:, c * BS + bt * 128: c * BS + (bt + 1) * 128],
                                  pt[:])
                      else:
                          ot = scr.tile([128, N], F32, tag="hn", bufs=3)
                          for h2 in range(2):
                              sl = slice(bt * N + h2 * 512, bt * N + (h2 + 1) * 512)
                              hs = slice(h2 * 512, (h2 + 1) * 512)
                              eng = nc.gpsimd if (bt, h2) in ((2, 1), (3, 1)) else nc.vector
                              eng.tensor_tensor(
                                  out=ot[:, hs], in0=z[:, sl], in1=rows_b[:, hs],
                                  op=Alu.mult)
                              eng.tensor_tensor(
                                  out=ot[:, hs], in0=ot[:, hs],
                                  in1=rows_b[:, N + h2 * 512: N + (h2 + 1) * 512],
                                  op=Alu.add)
                              eng.tensor_tensor(
                                  out=ot[:, hs], in0=ot[:, hs],
                                  in1=xn[:, bt * N + h2 * 512: bt * N + (h2 + 1) * 512],
                                  op=Alu.add)
                          nc.sync.dma_start(out[bt * 128:(bt + 1) * 128, :], ot[:])
            if loop_reps:
                with tc.For_i(0, loop_reps, 1):
                    body()
            else:
                for _rep in range(reps):
                    body()

    nc.compile()
    return nc


def _host_prep(x, mu1, lam1, v1, g1, b1, mu2, lam2, v2, g2, b2):
    """Build the device-input arrays (all float32, transposed on host)."""
    def build_w(mu, lam_, v):
        mu64 = mu.astype(np.float64)
        v64 = v.astype(np.float64)
        lam64 = lam_.astype(np.float64)
        W = np.empty((WROWS, WCOLS), np.float32)
        W[:D, :N] = (-2.0 * lam64[:, None] * mu64).T.astype(np.float32)
        W[:D, N:] = v.reshape(N * K, D).T.astype(np.float32)
        W[D, :N] = (lam64 * (mu64 * mu64).sum(1)).astype(np.float32)
        W[D, N:] = (-(v64 * mu64[:, None, :]).sum(-1)).reshape(-1).astype(np.float32)
        return W

    W1 = build_w(mu1, lam1, v1)
    W2 = build_w(mu2, lam2, v2)
    xT = np.ascontiguousarray(x.T)
    xsq = (x.astype(np.float64) ** 2).sum(1).astype(np.float32)
    lam_rows = np.stack([lam1, lam2]).astype(np.float32)
    cst = np.eye(128, dtype=np.float32)
    gb_rows = np.stack([g1, b1, g2, b2]).astype(np.float32)

    in_maps = []
    for c in range(N_CORES):
        rs = slice(c * BS, (c + 1) * BS)
        in_maps.append({
            "xT_s": np.ascontiguousarray(xT[:, rs]),
            "xn_s": np.ascontiguousarray(x[rs]),
            "xsq_s": np.ascontiguousarray(xsq[rs].reshape(1, BS)),
            "W1": W1, "W2": W2,
            "lam": lam_rows, "gb": gb_rows, "cst": cst,
            "ones_r": np.ones((1, 128), np.float32),
            "ones_c": np.ones((128, 1), np.float32),
        })
    return in_maps


def kernel(x, mu1, lam1, v1, g1, b1, mu2, lam2, v2, g2, b2):
    if "nc" not in _CACHE:
        _CACHE["nc"] = _build_nc()
    nc = _CACHE["nc"]
    in_maps = _host_prep(x, mu1, lam1, v1, g1, b1, mu2, lam2, v2, g2, b2)
    res = run_bass_kernel_spmd(nc, in_maps, list(range(N_CORES)))
    return np.concatenate([res.results[c]["out"] for c in range(N_CORES)], axis=0)



# revision 10
# speedup vs baseline: 125.4994x; 125.4994x over previous
"""Trainium2 Bass kernel for nn_BasicBlock_HMU (two HMU layers + sync BN + residual).

Sharding: data-parallel over batch (8 cores x 512 rows); params replicated.
BN batch statistics are all-gathered across the 8 cores (sync BN).

v2 design — n-on-partitions orientation, bf16 matmuls:
  Each layer computes quad^T [n_p, b_f] directly:
    quad = lam_n*|x_b|^2 - 2*lam_n*(mu.x) + lam_n*|mu|^2 + sum_k (v_k.x - v_k.mu)^2
  - weights are the stationary operand (streamed/resident bf16), x^T / h^T the
    moving operand; all per-n constants fold into activation BIAS columns
    (per-partition), so no constants-row matmuls.
  - lam_n*|x|^2 is one K=1 matmul per 128-n tile (8/layer).
  - per-k projection squares via Act Square (bias = -v.mu), summed on
    Pool+DVE, exp via Act with accum_out giving BN stat S1 for free; S2 via a
    second Act Square(bias=-1) accum_out. (z = e-1 shift keeps stats in fp32
    range; never materialized.)
  - layer-1 output [n_p, b_f] is ALREADY layer-2's moving operand: no PE
    transposes anywhere. Final output is written n-major and transposed on
    the host.
  - sync BN: stats [128,16] -> DRAM -> AllGather(8KB) -> strided DVE reduce,
    then per-partition affine (scale s, bias u) applied by DVE/Pool
    tensor_scalar ops; |h|^2 row for layer 2 via Act Square + ones-matmul
    partition reduction.
  - W2 is fully SBUF-resident (80KB/partition bf16), prefetched during
    layer 1 so layer 2 never touches HBM for weights.
"""

import numpy as np
import ml_dtypes

import concourse.bacc as bacc
import concourse.mybir as mybir
import concourse.tile as tile

try:
    from concourse.bass_utils import run_bass_kernel_spmd
except ImportError:  # pragma: no cover
    from bass_utils import run_bass_kernel_spmd

F32 = mybir.dt.float32
BF16 = mybir.dt.bfloat16
Alu = mybir.AluOpType
Act = mybir.ActivationFunctionType
BF = ml_dtypes.bfloat16

N_CORES = 8
B, D, N, K = 4096, 1024, 1024, 4
BS = B // N_CORES          # 512 rows per core
NT = N // 128              # 8 n-tiles per layer
CH = D // 128              # 8 contraction chunks
NU = NT * (1 + K)          # 40 stationary tiles per layer
WCOL = NU * 1024           # 40960 packed weight columns
BN_EPS = 1e-5
C1 = 1024.0 / 3.0          # host-side shift of the |x|^2 row (bf16 precision)

_CACHE = {}


def _build_nc(reps=1, loop_reps=0, collectives=True):
    nc = bacc.Bacc("TRN2", target_bir_lowering=False, debug=False,
                   num_devices=N_CORES)

    xTe_s = nc.dram_tensor("xTe_s", [128, CH * BS], BF16, kind="ExternalInput").ap()
    sqr_s = nc.dram_tensor("sqr_s", [1, BS], BF16, kind="ExternalInput").ap()
    W1p = nc.dram_tensor("W1p", [128, WCOL], BF16, kind="ExternalInput").ap()
    W2p = nc.dram_tensor("W2p", [128, WCOL], BF16, kind="ExternalInput").ap()
    lam2_s = nc.dram_tensor("lam2_s", [1, 2 * N], BF16, kind="ExternalInput").ap()
    cexp_s = nc.dram_tensor("cexp_s", [128, 16], F32, kind="ExternalInput").ap()
    cv_s = nc.dram_tensor("cv_s", [128, 64], F32, kind="ExternalInput").ap()
    gb_s = nc.dram_tensor("gb_s", [128, 32], F32, kind="ExternalInput").ap()
    ones_s = nc.dram_tensor("ones_s", [128, 1], BF16, kind="ExternalInput").ap()
    outT = nc.dram_tensor("outT", [N, BS], F32, kind="ExternalOutput").ap()

    with tile.TileContext(nc) as tc:
        with (
            tc.tile_pool(name="const", bufs=1) as constp,
            tc.tile_pool(name="big", bufs=1) as bigp,
            tc.tile_pool(name="wp", bufs=3) as wp,
            tc.tile_pool(name="scr", bufs=2) as scr,
            tc.tile_pool(name="rowp", bufs=1) as rowp,
            tc.tile_pool(name="fin", bufs=2) as finp,
            tc.tile_pool(name="pq", bufs=2, space="PSUM") as pq,
            tc.tile_pool(name="pp", bufs=4, space="PSUM") as pp,
            tc.tile_pool(name="ph", bufs=1, space="PSUM") as php,
            tc.tile_pool(name="dram", bufs=2, space="DRAM") as dramp,
        ):
            # ---- constants (loaded once, shared across reps) ----
            sqr = constp.tile([1, BS], BF16)
            nc.scalar.dma_start(sqr[:], sqr_s)
            lamt = constp.tile([1, 2 * N], BF16)
            nc.scalar.dma_start(lamt[:], lam2_s)
            cexp = constp.tile([128, 16], F32)
            nc.scalar.dma_start(cexp[:], cexp_s)
            cv = constp.tile([128, 64], F32)
            nc.scalar.dma_start(cv[:], cv_s)
            gb = constp.tile([128, 32], F32)
            nc.scalar.dma_start(gb[:], gb_s)
            onesc = constp.tile([128, 1], BF16)
            nc.scalar.dma_start(onesc[:], ones_s)
            epsc = constp.tile([128, 1], F32)
            nc.gpsimd.memset(epsc[:], BN_EPS)
            negc = constp.tile([128, 1], F32)
            nc.gpsimd.memset(negc[:], -1.0)
            warm = constp.tile([1, 1], F32)

            def body():
                xte = bigp.tile([128, CH * BS], BF16, tag="xte")
                nc.scalar.dma_start(xte[:], xTe_s)
                w2r = bigp.tile([128, WCOL], BF16, tag="w2r")
                hbf = bigp.tile([128, NT * BS], BF16, tag="hbf")
                hsqrow = rowp.tile([1, BS], BF16, tag="hsqrow")

                for L in range(2):
                    mv = (xte, hbf)[L]
                    srow = (sqr, hsqrow)[L]
                    e_all = bigp.tile([128, NT * BS], F32, tag="e")
                    stats = rowp.tile([128, 16], F32, tag="stats")

                    # ---- sweep: per n-tile, mu part then 4 v parts ----
                    for j in range(NT):
                        if L == 0:
                            wt = wp.tile([128, (1 + K) * 1024], BF16, tag="w")
                            nc.sync.dma_start(
                                wt[:], W1p[:, j * 5120:(j + 1) * 5120])
                        else:
                            wt = w2r[:, j * 5120:(j + 1) * 5120]
                        q = pq.tile([128, BS], F32, tag="q")
                        for c in range(CH):
                            nc.tensor.matmul(
                                q[:], wt[:, c * 128:(c + 1) * 128],
                                mv[:, c * BS:(c + 1) * BS],
                                start=(c == 0), stop=False)
                        nc.tensor.matmul(
                            q[:], lamt[0:1, L * N + j * 128: L * N + (j + 1) * 128],
                            srow[:], start=False, stop=True)
                        sqk = []
                        for k in range(K):
                            woff = (1 + k) * 1024
                            p = pp.tile([128, BS], F32, tag="p")
                            for c in range(CH):
                                nc.tensor.matmul(
                                    p[:], wt[:, woff + c * 128: woff + (c + 1) * 128],
                                    mv[:, c * BS:(c + 1) * BS],
                                    start=(c == 0), stop=(c == CH - 1))
                            sk = scr.tile([128, BS], F32, tag="sq", bufs=8)
                            ci = L * 32 + j * 4 + k
                            nc.scalar.activation(sk[:], p[:], Act.Square,
                                                 bias=cv[:, ci:ci + 1])
                            sqk.append(sk)
                        s01 = scr.tile([128, BS], F32, tag="s01")
                        nc.gpsimd.tensor_tensor(out=s01[:], in0=sqk[0][:],
                                                in1=sqk[1][:], op=Alu.add)
                        s23 = scr.tile([128, BS], F32, tag="s23")
                        nc.vector.tensor_tensor(out=s23[:], in0=sqk[2][:],
                                                in1=sqk[3][:], op=Alu.add)
                        s03 = scr.tile([128, BS], F32, tag="s03")
                        nc.gpsimd.tensor_tensor(out=s03[:], in0=s01[:],
                                                in1=s23[:], op=Alu.add)
                        qf = scr.tile([128, BS], F32, tag="qf")
                        nc.vector.tensor_tensor(out=qf[:], in0=q[:],
                                                in1=s03[:], op=Alu.add)
                        ej = e_all[:, j * BS:(j + 1) * BS]
                        nc.scalar.activation(ej, qf[:], Act.Exp,
                                             scale=-1.0 / D,
                                             bias=cexp[:, L * 8 + j: L * 8 + j + 1],
                                             accum_out=stats[:, j:j + 1])
                        scrq = scr.tile([128, BS], F32, tag="scrq")
                        nc.scalar.activation(scrq[:], ej, Act.Square,
                                             bias=negc[:],
                                             accum_out=stats[:, 8 + j:9 + j])
                        if L == 0:
                            # spread the resident-W2 prefetch across the sweep
                            nc.gpsimd.dma_start(
                                w2r[:, j * 5120:(j + 1) * 5120],
                                W2p[:, j * 5120:(j + 1) * 5120])

                    # ---- sync BN: stats -> AllGather -> reduce ----
                    cin = dramp.tile([128, 16], F32, tag="cin")
                    nc.scalar.dma_start(cin[:], stats[:])
                    cout = dramp.tile([N_CORES * 128, 16], F32, tag="cout",
                                      addr_space="Shared")
                    if collectives:
                        nc.gpsimd.collective_compute(
                            "AllGather", Alu.bypass,
                            replica_groups=[list(range(N_CORES))],
                            ins=[cin[:].opt()], outs=[cout[:].opt()])
                    else:
                        nc.sync.dma_start(cout[0:128, :], cin[:])
                    # preload the ACT Sqrt table off the critical path
                    nc.scalar.activation(warm[:], epsc[0:1, 0:1], Act.Sqrt)
                    gath = rowp.tile([128, N_CORES * 16], F32, tag="gath")
                    for g in range(N_CORES):
                        nc.scalar.dma_start(
                            gath[:, g * 16:(g + 1) * 16],
                            cout[g * 128:(g + 1) * 128, :])
                    red = rowp.tile([128, 16], F32, tag="red")
                    nc.vector.tensor_reduce(
                        out=red[:],
                        in_=gath[:].rearrange("p (g f) -> p f g", g=N_CORES),
                        axis=mybir.AxisListType.X, op=Alu.add)

                    # ---- finalize: s = g*rsqrt(var+eps), u = b - s*mean ----
                    m_e = finp.tile([128, 8], F32, tag="m_e")
                    nc.vector.tensor_scalar(out=m_e[:], in0=red[:, 0:8],
                                            scalar1=1.0 / B, scalar2=None,
                                            op0=Alu.mult)
                    mz = finp.tile([128, 8], F32, tag="mz")
                    nc.vector.tensor_scalar(out=mz[:], in0=m_e[:],
                                            scalar1=-1.0, scalar2=None,
                                            op0=Alu.add)
                    mz2 = finp.tile([128, 8], F32, tag="mz2")
                    nc.vector.tensor_tensor(out=mz2[:], in0=mz[:], in1=mz[:],
                                            op=Alu.mult)
                    varr = finp.tile([128, 8], F32, tag="varr")
                    nc.vector.scalar_tensor_tensor(
                        out=varr[:], in0=red[:, 8:16], scalar=1.0 / B,
                        in1=mz2[:], op0=Alu.mult, op1=Alu.subtract)
                    sd = finp.tile([128, 8], F32, tag="sd")
                    nc.scalar.activation(sd[:], varr[:], Act.Sqrt, bias=epsc[:])
                    rs = finp.tile([128, 8], F32, tag="rs")
                    nc.vector.reciprocal(rs[:], sd[:])
                    s_t = finp.tile([128, 8], F32, tag="s_t")
                    nc.vector.tensor_tensor(out=s_t[:], in0=rs[:],
                                            in1=gb[:, 16 * L:16 * L + 8],
                                            op=Alu.mult)
                    um = finp.tile([128, 8], F32, tag="um")
                    nc.vector.tensor_tensor(out=um[:], in0=s_t[:], in1=m_e[:],
                                            op=Alu.mult)
                    u_t = finp.tile([128, 8], F32, tag="u_t")
                    nc.vector.tensor_tensor(out=u_t[:],
                                            in0=gb[:, 16 * L + 8:16 * L + 16],
                                            in1=um[:], op=Alu.subtract)

                    # ---- normalize (+ |h|^2 row | + residual & store) ----
                    if L == 0:
                        hsqp = php.tile([1, BS], F32, tag="hsq")
                        for j in range(NT):
                            eng = (nc.vector, nc.gpsimd)[j % 2]
                            eng.tensor_scalar(
                                out=hbf[:, j * BS:(j + 1) * BS],
                                in0=e_all[:, j * BS:(j + 1) * BS],
                                scalar1=s_t[:, j:j + 1], scalar2=u_t[:, j:j + 1],
                                op0=Alu.mult, op1=Alu.add)
                            hh = scr.tile([128, BS], BF16, tag="hh")
                            nc.scalar.activation(
                                hh[:], e_all[:, j * BS:(j + 1) * BS], Act.Square,
                                scale=s_t[:, j:j + 1], bias=u_t[:, j:j + 1])
                            nc.tensor.matmul(hsqp[:], onesc[:], hh[:],
                                             start=(j == 0), stop=(j == NT - 1))
                        nc.scalar.copy(hsqrow[:], hsqp[:])
                    else:
                        for j in range(NT):
                            ot = scr.tile([128, BS], F32, tag="ot", bufs=3)
                            e0, e1 = ((nc.vector, nc.gpsimd),
                                      (nc.gpsimd, nc.vector))[j % 2]
                            e0.tensor_scalar(
                                out=ot[:], in0=e_all[:, j * BS:(j + 1) * BS],
                                scalar1=s_t[:, j:j + 1], scalar2=u_t[:, j:j + 1],
                                op0=Alu.mult, op1=Alu.add)
                            e1.tensor_tensor(
                                out=ot[:], in0=ot[:],
                                in1=xte[:, j * BS:(j + 1) * BS], op=Alu.add)
                            nc.sync.dma_start(outT[j * 128:(j + 1) * 128, :], ot[:])

            if loop_reps:
                with tc.For_i(0, loop_reps, 1):
                    body()
            else:
                for _rep in range(reps):
                    body()

    nc.compile()
    return nc


def _host_prep(x, mu1, lam1, v1, g1, b1, mu2, lam2, v2, g2, b2):
    """Build the device-input arrays (bf16 weights/activations, f32 consts)."""
    def pack_layer(mu, lam_, v, c_shift):
        mu64 = mu.astype(np.float64)
        v64 = v.astype(np.float64)
        lam64 = lam_.astype(np.float64)
        Wmu = (-2.0 * lam64[:, None] * mu64).T            # [D, N]
        Wv = v64.transpose(1, 0, 2).reshape(K * N, D).T   # [D, K*N] k-major
        blocks = []
        for j in range(NT):
            blocks.append(Wmu[:, j * 128:(j + 1) * 128])
            for k in range(K):
                blocks.append(Wv[:, k * N + j * 128: k * N + (j + 1) * 128])
        Wp = np.concatenate(
            [b.reshape(CH, 128, 128).transpose(1, 0, 2).reshape(128, 1024)
             for b in blocks], axis=1).astype(BF)         # [128, WCOL]
        vm = (v64 * mu64[:, None, :]).sum(-1)             # [N, K]
        cv_l = (-vm).reshape(NT, 128, K).transpose(1, 0, 2).reshape(128, NT * K)
        musq = (mu64 * mu64).sum(1)
        ce = (-(lam64 * (musq + c_shift)) / D).reshape(NT, 128).T
        return Wp, cv_l.astype(np.float32), ce.astype(np.float32)

    W1pk, cv1, ce1 = pack_layer(mu1, lam1, v1, C1)
    W2pk, cv2, ce2 = pack_layer(mu2, lam2, v2, 0.0)
    cv_all = np.concatenate([cv1, cv2], axis=1)               # [128, 64]
    cexp = np.concatenate([ce1, ce2], axis=1)                 # [128, 16]
    gbp = np.concatenate(
        [a.reshape(NT, 128).T for a in (g1, b1, g2, b2)],
        axis=1).astype(np.float32)                            # [128, 32]
    lam_bf = np.concatenate([lam1, lam2]).reshape(1, 2 * N).astype(BF)

    x64 = x.astype(np.float64)
    xT = np.ascontiguousarray(x.T)                            # [D, B]
    xsq = (x64 * x64).sum(1) - C1                             # [B]

    in_maps = []
    for c in range(N_CORES):
        rs = slice(c * BS, (c + 1) * BS)
        xte = (xT[:, rs].reshape(CH, 128, BS).transpose(1, 0, 2)
               .reshape(128, CH * BS)).astype(BF)
        in_maps.append({
            "xTe_s": xte,
            "sqr_s": xsq[rs].reshape(1, BS).astype(BF),
            "W1p": W1pk, "W2p": W2pk,
            "lam2_s": lam_bf, "cexp_s": cexp, "cv_s": cv_all, "gb_s": gbp,
            "ones_s": np.ones((128, 1), BF),
        })
    return in_maps


def kernel(x, mu1, lam1, v1, g1, b1, mu2, lam2, v2, g2, b2):
    if "nc" not in _CACHE:
        _CACHE["nc"] = _build_nc()
    nc = _CACHE["nc"]
    in_maps = _host_prep(x, mu1, lam1, v1, g1, b1, mu2, lam2, v2, g2, b2)
    res = run_bass_kernel_spmd(nc, in_maps, list(range(N_CORES)))
    return np.concatenate(
        [res.results[c]["outT"].T for c in range(N_CORES)], axis=0)
